# revision 31
# baseline (speedup 1.0000x reference)
"""Duration-based length regulation (KittenTTS LengthRegulator) on 8 trn2 NeuronCores.

For each batch b (one per core): phoneme t's feature row is repeated
clamp(durations[b,t],1) times along the frame axis; frames are zero-padded to
MAX_LEN = T*15 (outputs arrive pre-zeroed from the runner, so padding rows are
simply left unwritten... except sink rows, sliced off host-side).

Device strategy (per core, batch-parallel across 8 cores):
  Partition p owns phonemes 4p..4p+3 (IPB=4), loaded as one contiguous DMA.
  Cumsum: per-partition DVE scan + strict-lower-triangular ones matmul on PE
  for the cross-partition exclusive prefix (no host-side index math).
  Expand via 16 single-rank indirect scatters ({8,4,2,1}-row binary blocks of
  dur x 4 phoneme slots; masked slots pushed past bounds_check and skipped),
  issued in a RAW bass region after TileContext:
  without Tile's conservative WAW waits between them, Q7 streams descriptor
  generation back-to-back and the SDMA drains overlap -- the v1 kernel lost
  ~50us to per-call completion stalls here. Write traffic ~= live bytes.
"""

import sys

import numpy as np

if "/opt/trn_rl_repo" not in sys.path:
    sys.path.insert(0, "/opt/trn_rl_repo")

B, T, D = 8, 512, 512
MAX_DUR = 15
MAX_LEN = T * MAX_DUR  # 7680
P = 128
IPB = T // P  # 4 phonemes per partition
BLKS = [8, 4, 2, 1]  # binary block sizes
OFFS_ORDER = [1, 8, 4, 2]  # offset column order (s=1 first)
OOB = 1 << 20

_CACHE = {}


def _build_nc():
    from concourse import bass, mybir
    from concourse.bacc import Bacc
    from concourse.tile import TileContext

    f32, i32, i16 = mybir.dt.float32, mybir.dt.int32, mybir.dt.int16
    Alu = mybir.AluOpType

    nc = Bacc()
    feats = nc.declare_dram_parameter("features", [T, D], f32, isOutput=False)
    durs = nc.declare_dram_parameter("durations", [P, IPB], i32, isOutput=False)
    out = nc.declare_dram_parameter("out", [MAX_LEN, D], f32, isOutput=True)
    
    cpy_sem = nc.alloc_semaphore("cpy")
    go_sem = nc.alloc_semaphore("go")
    scat_sem = nc.alloc_semaphore("scat")

    with TileContext(nc) as tc:
        with (
            tc.tile_pool(name="sbuf", bufs=1) as sb,
            tc.tile_pool(name="psum", bufs=1, space="PSUM") as pp,
        ):
            # --- durations [128, 4]: dur[p, i] = durations[4p+i], clamped >= 1
            dur = sb.tile([P, IPB], i32, tag="dur")
            nc.sync.dma_start(out=dur[:], in_=durs[:, :])
            nc.vector.tensor_scalar_max(out=dur[:], in0=dur[:], scalar1=1)

            # --- features: fB[p, i*D:(i+1)*D] = feat[4p+i, :]
            fB_h = nc.alloc_sbuf_tensor("fB_raw", [P, IPB * D], f32)
            fB = fB_h
            nc.sync.dma_start(
                out=fB[:],
                in_=feats[:, :].rearrange("(p i) d -> p (i d)", p=P),
            )
            fBv = fB[:].rearrange("p (i d) -> p i d", i=IPB)

            # dummy ACT op: forces the activation table load into the preamble
            # instead of stalling the first raw-region scalar copy
            warm = sb.tile([P, 1], f32, tag="warm")
            nc.vector.memset(warm[:], 0.0)
            nc.scalar.copy(out=warm[:], in_=warm[:])

            # --- strict-lower-triangular ones [128, 128]: ltri[k, p] = (p > k)
            iota_f = sb.tile([P, P], i32, tag="iota_f")
            nc.gpsimd.iota(out=iota_f[:], pattern=[[1, P]], base=0, channel_multiplier=0)
            iota_p = sb.tile([P, 1], i32, tag="iota_p")
            nc.gpsimd.iota(out=iota_p[:], pattern=[[1, 1]], base=0, channel_multiplier=1)
            ltri = sb.tile([P, P], f32, tag="ltri")
            nc.vector.tensor_tensor(
                out=ltri[:],
                in0=iota_f[:],
                in1=iota_p[:, 0:1].to_broadcast([P, P]),
                op=Alu.is_gt,
            )

            # --- within-partition inclusive scan of the 4 durations
            scan = sb.tile([P, IPB], i32, tag="scan")
            nc.vector.tensor_tensor_scan(
                out=scan[:],
                data0=dur[:],
                data1=dur[:],
                initial=0.0,
                op0=Alu.add,
                op1=Alu.bypass,
            )

            # --- cross-partition exclusive prefix of per-partition totals
            sf = sb.tile([P, 1], f32, tag="sf")
            nc.vector.tensor_copy(out=sf[:], in_=scan[:, IPB - 1 : IPB])
            pre_ps = pp.tile([P, 1], f32, tag="pre_ps")
            nc.tensor.matmul(pre_ps[:], ltri[:], sf[:], start=True, stop=True)
            pre = sb.tile([P, 1], i32, tag="pre")
            nc.vector.tensor_copy(out=pre[:], in_=pre_ps[:])

            # exc[p, i] = global exclusive cumsum at phoneme 4p+i
            exc = sb.tile([P, IPB], i32, tag="exc")
            nc.vector.tensor_tensor(out=exc[:], in0=scan[:], in1=dur[:], op=Alu.subtract)
            nc.vector.tensor_tensor(
                out=exc[:], in0=exc[:], in1=pre[:, 0:1].to_broadcast([P, IPB]), op=Alu.add
            )

            # --- scatter offsets [128, 16], col ci*4+i: binary block of size
            # s at exc + (dur & ~(2s-1)), masked to OOB unless (dur & s).
            # Computed as wide [128, 16] ops against per-column constant tiles
            # (built early on Pool). Column class order: OFFS_ORDER.
            offs_h = nc.alloc_sbuf_tensor("offs_raw", [P, 16], i32)
            offs = offs_h[:]
            negm = sb.tile([P, 16], i32, tag="negm")
            smask = sb.tile([P, 16], i32, tag="smask")
            for ci, s_ in enumerate(OFFS_ORDER):
                cols = slice(ci * IPB, (ci + 1) * IPB)
                nc.gpsimd.iota(out=negm[:, cols], pattern=[[0, IPB]], base=-(2 * s_), channel_multiplier=0)
                nc.gpsimd.iota(out=smask[:, cols], pattern=[[0, IPB]], base=s_, channel_multiplier=0)
            dur16 = sb.tile([P, 16], i32, tag="dur16")
            nc.vector.tensor_copy(
                out=dur16[:].rearrange("p (a i) -> p a i", a=IPB),
                in_=dur[:, None, :].to_broadcast([P, IPB, IPB]),
            )
            exc16 = sb.tile([P, 16], i32, tag="exc16")
            nc.vector.tensor_copy(
                out=exc16[:].rearrange("p (a i) -> p a i", a=IPB),
                in_=exc[:, None, :].to_broadcast([P, IPB, IPB]),
            )
            m16 = sb.tile([P, 16], i32, tag="m16")
            nc.vector.tensor_tensor(out=offs[:, :], in0=dur16[:], in1=negm[:], op=Alu.bitwise_and)
            nc.vector.tensor_tensor(out=offs[:, :], in0=offs[:, :], in1=exc16[:], op=Alu.add)
            nc.vector.tensor_tensor(out=m16[:], in0=dur16[:], in1=smask[:], op=Alu.bitwise_and)
            nc.vector.tensor_scalar(
                out=m16[:], in0=m16[:], scalar1=0, scalar2=OOB, op0=Alu.is_equal, op1=Alu.mult
            )
            nc.vector.tensor_tensor(out=offs[:, :], in0=offs[:, :], in1=m16[:], op=Alu.add)

            # rep[p, i, c, :] = feat[4p+i, :] for c = 0..3; built in the RAW
            # region (below) so replication overlaps scatter descriptor-gen.
            rep_h = nc.alloc_sbuf_tensor("rep_raw", [P, IPB, 8, D], f32)
            rep = rep_h[:]

    # --- RAW region. The TileContext epilogue barrier guarantees fB, dur and
    # offs are ready. Copies (DVE: phonemes 0-1, ACT: 2-3) run concurrently
    # with the Pool engine's descriptor generation for the copy-free 1-row
    # scatters; the remaining scatters gate on the copy semaphore. No WAW
    # waits between scatters: Q7 streams desc-gen, SDMA drains overlap.
    fBr = fB[:].rearrange("p (i d) -> p i d", i=IPB)
    # The TileContext epilogue ends with a Pool-engine semaphore-range CLEAR;
    # raw-region sem increments on other engines must not race it. Pool
    # publishes go_sem after its epilogue; DVE/ACT wait before their copies.
    nc.gpsimd.sem_inc(go_sem, 1)
    nc.vector.wait_ge(go_sem, 1)
    nc.scalar.wait_ge(go_sem, 1)
    for c in range(8):
        v = nc.vector.tensor_copy(out=rep[:, 0:2, c, :], in_=fBr[:, 0:2, :])
        if c < 6:
            a = nc.scalar.copy(out=rep[:, 2:4, c, :], in_=fBr[:, 2:4, :])
            if c == 5:
                a.then_inc(cpy_sem, 1)
    nc.vector.tensor_copy(out=rep[:, 2:4, 6, :], in_=fBr[:, 2:4, :])
    nc.vector.tensor_copy(out=rep[:, 2:4, 7, :], in_=fBr[:, 2:4, :]).then_inc(cpy_sem, 1)

    bregs = {s_: nc.gpsimd.to_reg(MAX_LEN - s_) for s_ in BLKS}

    def scat(ci, i, src_ap, s_):
        c = ci * IPB + i
        nc.gpsimd.indirect_dma_start(
            out=out[:, :],
            out_offset=bass.IndirectOffsetOnAxis(ap=offs[:, c : c + 1], axis=0),
            in_=src_ap,
            in_offset=None,
            bounds_check=bregs[s_],
            oob_is_err=False,
        ).then_inc(scat_sem, 16)

    for i in range(IPB):  # 1-row blocks: source is fB, no copy dependency
        scat(0, i, fBr[:, i, :], 1)
    nc.gpsimd.wait_ge(cpy_sem, 2)
    for ci, s_ in ((1, 8), (2, 4), (3, 2)):
        for i in range(IPB):
            scat(ci, i, rep[:, i, 0:s_, :].rearrange("p c d -> p (c d)"), s_)
    nc.gpsimd.wait_ge(scat_sem, 16 * 16)

    nc.compile()
    return nc


def _get_nc():
    if "nc" not in _CACHE:
        _CACHE["nc"] = _build_nc()
    return _CACHE["nc"]


def _run(features, durations, trace=False):
    """features (B,T,D) f32, durations (B,T) i32 -> (out (B,MAX_LEN,D) f32, BassKernelResults)."""
    from concourse.bass_utils import run_bass_kernel_spmd

    nc = _get_nc()
    in_maps = []
    for b in range(B):
        in_maps.append(
            {
                "features": np.ascontiguousarray(features[b]),
                "durations": np.ascontiguousarray(durations[b].reshape(P, IPB)),
            }
        )
    kwargs = {}
    if trace:
        kwargs = dict(trace=True, trace_cores=list(range(B)), stitch_traces=False)
    res = run_bass_kernel_spmd(nc, in_maps, core_ids=list(range(B)), **kwargs)
    outs = np.stack([res.results[b]["out"] for b in range(B)])
    return outs.astype(np.float32, copy=False), res


def kernel(features, durations):
    features = np.asarray(features, dtype=np.float32)
    durations = np.asarray(durations, dtype=np.int32)
    outs, _ = _run(features, durations, trace=False)
    return outs


if __name__ == "__main__":
    feats = np.random.randn(B, T, D).astype(np.float32)
    durs = np.random.randint(0, 16, size=(B, T)).astype(np.int32)
    out = kernel(feats, durs)
    print("out", out.shape, out.dtype)


# revision 32
# speedup vs baseline: 1.0560x; 1.0560x over previous
"""Duration-based length regulation (KittenTTS LengthRegulator) on 8 trn2 NeuronCores.

For each batch b (one per core): phoneme t's feature row is repeated
clamp(durations[b,t],1) times along the frame axis; frames are zero-padded to
MAX_LEN = T*15 (outputs arrive pre-zeroed from the runner, so padding rows are
simply left unwritten... except sink rows, sliced off host-side).

Device strategy (per core, batch-parallel across 8 cores):
  Partition p owns phonemes 4p..4p+3 (IPB=4), loaded as one contiguous DMA.
  Cumsum: per-partition DVE scan + strict-lower-triangular ones matmul on PE
  for the cross-partition exclusive prefix (no host-side index math).
  Expand via 16 single-rank indirect scatters ({8,4,2,1}-row binary blocks of
  dur x 4 phoneme slots; masked slots pushed past bounds_check and skipped),
  issued in a RAW bass region after TileContext:
  without Tile's conservative WAW waits between them, Q7 streams descriptor
  generation back-to-back and the SDMA drains overlap -- the v1 kernel lost
  ~50us to per-call completion stalls here. Write traffic ~= live bytes.
"""

import sys

import numpy as np

if "/opt/trn_rl_repo" not in sys.path:
    sys.path.insert(0, "/opt/trn_rl_repo")

B, T, D = 8, 512, 512
MAX_DUR = 15
MAX_LEN = T * MAX_DUR  # 7680
P = 128
IPB = T // P  # 4 phonemes per partition
BLKS = [8, 4, 2, 1]  # binary block sizes
OFFS_ORDER = [1, 8, 4, 2]  # offset column order (s=1 first)
OOB = 1 << 20

_CACHE = {}


def _build_nc():
    from concourse import bass, mybir
    from concourse.bacc import Bacc
    from concourse.tile import TileContext

    f32, i32, i16 = mybir.dt.float32, mybir.dt.int32, mybir.dt.int16
    Alu = mybir.AluOpType

    nc = Bacc()
    feats = nc.declare_dram_parameter("features", [T, D], f32, isOutput=False)
    durs = nc.declare_dram_parameter("durations", [P, IPB], i32, isOutput=False)
    out = nc.declare_dram_parameter("out", [MAX_LEN, D], f32, isOutput=True)
    
    cpy_sem = nc.alloc_semaphore("cpy")
    go_sem = nc.alloc_semaphore("go")
    scat_sem = nc.alloc_semaphore("scat")

    with TileContext(nc) as tc:
        with (
            tc.tile_pool(name="sbuf", bufs=1) as sb,
            tc.tile_pool(name="psum", bufs=1, space="PSUM") as pp,
        ):
            # --- durations [128, 4]: dur[p, i] = durations[4p+i], clamped >= 1
            dur = sb.tile([P, IPB], i32, tag="dur")
            nc.sync.dma_start(out=dur[:], in_=durs[:, :])
            nc.vector.tensor_scalar_max(out=dur[:], in0=dur[:], scalar1=1)

            # --- features: fB[p, i*D:(i+1)*D] = feat[4p+i, :]
            fB_h = nc.alloc_sbuf_tensor("fB_raw", [P, IPB * D], f32)
            fB = fB_h
            nc.sync.dma_start(
                out=fB[:],
                in_=feats[:, :].rearrange("(p i) d -> p (i d)", p=P),
            )
            fBv = fB[:].rearrange("p (i d) -> p i d", i=IPB)

            # dummy ACT op: forces the activation table load into the preamble
            # instead of stalling the first raw-region scalar copy
            warm = sb.tile([P, 1], f32, tag="warm")
            nc.vector.memset(warm[:], 0.0)
            nc.scalar.copy(out=warm[:], in_=warm[:])

            # --- strict-lower-triangular ones [128, 128]: ltri[k, p] = (p > k)
            iota_f = sb.tile([P, P], i32, tag="iota_f")
            nc.gpsimd.iota(out=iota_f[:], pattern=[[1, P]], base=0, channel_multiplier=0)
            iota_p = sb.tile([P, 1], i32, tag="iota_p")
            nc.gpsimd.iota(out=iota_p[:], pattern=[[1, 1]], base=0, channel_multiplier=1)
            ltri = sb.tile([P, P], f32, tag="ltri")
            nc.vector.tensor_tensor(
                out=ltri[:],
                in0=iota_f[:],
                in1=iota_p[:, 0:1].to_broadcast([P, P]),
                op=Alu.is_gt,
            )

            # --- within-partition inclusive scan of the 4 durations
            scan = sb.tile([P, IPB], i32, tag="scan")
            nc.vector.tensor_tensor_scan(
                out=scan[:],
                data0=dur[:],
                data1=dur[:],
                initial=0.0,
                op0=Alu.add,
                op1=Alu.bypass,
            )

            # --- cross-partition exclusive prefix of per-partition totals
            sf = sb.tile([P, 1], f32, tag="sf")
            nc.vector.tensor_copy(out=sf[:], in_=scan[:, IPB - 1 : IPB])
            pre_ps = pp.tile([P, 1], f32, tag="pre_ps")
            nc.tensor.matmul(pre_ps[:], ltri[:], sf[:], start=True, stop=True)
            pre = sb.tile([P, 1], i32, tag="pre")
            nc.vector.tensor_copy(out=pre[:], in_=pre_ps[:])

            # exc[p, i] = global exclusive cumsum at phoneme 4p+i
            exc = sb.tile([P, IPB], i32, tag="exc")
            nc.vector.tensor_tensor(out=exc[:], in0=scan[:], in1=dur[:], op=Alu.subtract)
            nc.vector.tensor_tensor(
                out=exc[:], in0=exc[:], in1=pre[:, 0:1].to_broadcast([P, IPB]), op=Alu.add
            )

            # --- scatter offsets [128, 16], col ci*4+i: binary block of size
            # s at exc + (dur & ~(2s-1)), masked to OOB unless (dur & s).
            # Computed as wide [128, 16] ops against per-column constant tiles
            # (built early on Pool). Column class order: OFFS_ORDER.
            offs_h = nc.alloc_sbuf_tensor("offs_raw", [P, 16], i32)
            offs = offs_h[:]
            negm = sb.tile([P, 16], i32, tag="negm")
            smask = sb.tile([P, 16], i32, tag="smask")
            for ci, s_ in enumerate(OFFS_ORDER):
                cols = slice(ci * IPB, (ci + 1) * IPB)
                nc.gpsimd.iota(out=negm[:, cols], pattern=[[0, IPB]], base=-(2 * s_), channel_multiplier=0)
                nc.gpsimd.iota(out=smask[:, cols], pattern=[[0, IPB]], base=s_, channel_multiplier=0)
            dur16 = sb.tile([P, 16], i32, tag="dur16")
            nc.vector.tensor_copy(
                out=dur16[:].rearrange("p (a i) -> p a i", a=IPB),
                in_=dur[:, None, :].to_broadcast([P, IPB, IPB]),
            )
            exc16 = sb.tile([P, 16], i32, tag="exc16")
            nc.vector.tensor_copy(
                out=exc16[:].rearrange("p (a i) -> p a i", a=IPB),
                in_=exc[:, None, :].to_broadcast([P, IPB, IPB]),
            )
            m16 = sb.tile([P, 16], i32, tag="m16")
            nc.vector.tensor_tensor(out=offs[:, :], in0=dur16[:], in1=negm[:], op=Alu.bitwise_and)
            nc.vector.tensor_tensor(out=offs[:, :], in0=offs[:, :], in1=exc16[:], op=Alu.add)
            nc.vector.tensor_tensor(out=m16[:], in0=dur16[:], in1=smask[:], op=Alu.bitwise_and)
            nc.vector.tensor_scalar(
                out=m16[:], in0=m16[:], scalar1=0, scalar2=OOB, op0=Alu.is_equal, op1=Alu.mult
            )
            nc.vector.tensor_tensor(out=offs[:, :], in0=offs[:, :], in1=m16[:], op=Alu.add)

            # rep[p, i, c, :] = feat[4p+i, :] for c = 0..7; built in the RAW
            # region (below) so replication overlaps scatter descriptor-gen.
            rep_h = nc.alloc_sbuf_tensor("rep_raw", [P, IPB, 8, D], f32)
            rep = rep_h[:]
            # bounds registers for the raw-region scatters (Pool is idle here)
            bregs = {s_: nc.gpsimd.to_reg(MAX_LEN - s_) for s_ in BLKS}

    # --- RAW region. The TileContext epilogue barrier guarantees fB, dur and
    # offs are ready. Copies (DVE: phonemes 0-1, ACT: 2-3) run concurrently
    # with the Pool engine's descriptor generation for the copy-free 1-row
    # scatters; the remaining scatters gate on the copy semaphore. No WAW
    # waits between scatters: Q7 streams desc-gen, SDMA drains overlap.
    fBr = fB[:].rearrange("p (i d) -> p i d", i=IPB)
    # The TileContext epilogue ends with a Pool-engine semaphore-range CLEAR;
    # raw-region sem increments on other engines must not race it. Pool
    # publishes go_sem after its epilogue; DVE/ACT wait before their copies.
    nc.gpsimd.sem_inc(go_sem, 1)
    nc.vector.wait_ge(go_sem, 1)
    nc.scalar.wait_ge(go_sem, 1)
    for c in range(8):
        v = nc.vector.tensor_copy(out=rep[:, 0:2, c, :], in_=fBr[:, 0:2, :])
        if c < 6:
            a = nc.scalar.copy(out=rep[:, 2:4, c, :], in_=fBr[:, 2:4, :])
            if c == 5:
                a.then_inc(cpy_sem, 1)
    nc.vector.tensor_copy(out=rep[:, 2:4, 6, :], in_=fBr[:, 2:4, :])
    nc.vector.tensor_copy(out=rep[:, 2:4, 7, :], in_=fBr[:, 2:4, :]).then_inc(cpy_sem, 1)

    def scat(ci, i, src_ap, s_):
        c = ci * IPB + i
        nc.gpsimd.indirect_dma_start(
            out=out[:, :],
            out_offset=bass.IndirectOffsetOnAxis(ap=offs[:, c : c + 1], axis=0),
            in_=src_ap,
            in_offset=None,
            bounds_check=bregs[s_],
            oob_is_err=False,
        ).then_inc(scat_sem, 16)

    for i in range(IPB):  # 1-row blocks: source is fB, no copy dependency
        scat(0, i, fBr[:, i, :], 1)
    nc.gpsimd.wait_ge(cpy_sem, 2)
    for ci, s_ in ((1, 8), (2, 4), (3, 2)):
        for i in range(IPB):
            scat(ci, i, rep[:, i, 0:s_, :].rearrange("p c d -> p (c d)"), s_)
    nc.gpsimd.wait_ge(scat_sem, 16 * 16)

    nc.compile()
    return nc


def _get_nc():
    if "nc" not in _CACHE:
        _CACHE["nc"] = _build_nc()
    return _CACHE["nc"]


def _run(features, durations, trace=False):
    """features (B,T,D) f32, durations (B,T) i32 -> (out (B,MAX_LEN,D) f32, BassKernelResults)."""
    from concourse.bass_utils import run_bass_kernel_spmd

    nc = _get_nc()
    in_maps = []
    for b in range(B):
        in_maps.append(
            {
                "features": np.ascontiguousarray(features[b]),
                "durations": np.ascontiguousarray(durations[b].reshape(P, IPB)),
            }
        )
    kwargs = {}
    if trace:
        kwargs = dict(trace=True, trace_cores=list(range(B)), stitch_traces=False)
    res = run_bass_kernel_spmd(nc, in_maps, core_ids=list(range(B)), **kwargs)
    outs = np.stack([res.results[b]["out"] for b in range(B)])
    return outs.astype(np.float32, copy=False), res


def kernel(features, durations):
    features = np.asarray(features, dtype=np.float32)
    durations = np.asarray(durations, dtype=np.int32)
    outs, _ = _run(features, durations, trace=False)
    return outs


if __name__ == "__main__":
    feats = np.random.randn(B, T, D).astype(np.float32)
    durs = np.random.randint(0, 16, size=(B, T)).astype(np.int32)
    out = kernel(feats, durs)
    print("out", out.shape, out.dtype)


# revision 33
# speedup vs baseline: 1.0580x; 1.0019x over previous
"""Duration-based length regulation (KittenTTS LengthRegulator) on 8 trn2 NeuronCores.

For each batch b (one per core): phoneme t's feature row is repeated
clamp(durations[b,t],1) times along the frame axis; frames are zero-padded to
MAX_LEN = T*15 (outputs arrive pre-zeroed from the runner, so padding rows are
simply left unwritten).

Device strategy (per core, batch-parallel across 8 cores; 102.6us -> ~48.7us):
  Partition p owns phonemes 4p..4p+3 (IPB=4), so features load as one
  contiguous DMA and durations as [128, 4] with no host-side transpose.
  Cumsum: per-partition DVE scan of the 4 local durations + a
  strict-lower-triangular ones matmul on PE for the cross-partition exclusive
  prefix (replaces v1's DRAM scratch round-trip). Offsets for all 16 scatter
  columns are computed as wide [128, 16] DVE ops against constant tiles.
  Expand: 16 single-rank indirect scatters ({8,4,2,1}-row binary blocks of
  dur x 4 phoneme slots; masked slots pushed past bounds_check via OOB and
  silently skipped, so HBM write traffic ~= live output bytes ~8MB/core).

  The scatters are issued in a RAW bass region after TileContext exits:
  Tile's conservative WAW tracking on `out` serialized each scatter behind
  the previous one's DMA completion (~5.5us/call, the v1 bottleneck); raw
  emission lets Q7 stream descriptor generation back-to-back (~1.4us/call)
  with SDMA drains fully overlapped. The 8 source replicas per phoneme are
  built in the raw region too (DVE/ACT, all copied from the load buffer to
  avoid read-after-write engine stalls), concurrent with the descriptor
  generation of the copy-free 1-row scatters; the wider scatters gate on a
  copy semaphore. A go-semaphore handshake keeps those cross-engine sem
  increments from racing the TileContext epilogue's semaphore-range clear.

  Indirect scatters here are strictly ONE offset per partition per call:
  the INDIRECT1D ucode's multi-offset source walk was observed (HW probing)
  to derive per-rank source offsets from the first index value
  (src_off(j) ~ j*(num_per_idx + idx0*D)) -- unusable for packed offsets.
"""

import sys

import numpy as np

if "/opt/trn_rl_repo" not in sys.path:
    sys.path.insert(0, "/opt/trn_rl_repo")

B, T, D = 8, 512, 512
MAX_DUR = 15
MAX_LEN = T * MAX_DUR  # 7680
P = 128
IPB = T // P  # 4 phonemes per partition
BLKS = [8, 4, 2, 1]  # binary block sizes
OFFS_ORDER = [1, 8, 4, 2]  # offset column order (s=1 first)
OOB = 1 << 20

_CACHE = {}


def _build_nc():
    from concourse import bass, mybir
    from concourse.bacc import Bacc
    from concourse.tile import TileContext

    f32, i32, i16 = mybir.dt.float32, mybir.dt.int32, mybir.dt.int16
    Alu = mybir.AluOpType

    nc = Bacc()
    feats = nc.declare_dram_parameter("features", [T, D], f32, isOutput=False)
    durs = nc.declare_dram_parameter("durations", [P, IPB], i32, isOutput=False)
    out = nc.declare_dram_parameter("out", [MAX_LEN, D], f32, isOutput=True)
    
    cpy_sem = nc.alloc_semaphore("cpy")
    go_sem = nc.alloc_semaphore("go")
    scat_sem = nc.alloc_semaphore("scat")

    with TileContext(nc) as tc:
        with (
            tc.tile_pool(name="sbuf", bufs=1) as sb,
            tc.tile_pool(name="psum", bufs=1, space="PSUM") as pp,
        ):
            # --- durations [128, 4]: dur[p, i] = durations[4p+i], clamped >= 1
            dur = sb.tile([P, IPB], i32, tag="dur")
            nc.sync.dma_start(out=dur[:], in_=durs[:, :])
            nc.vector.tensor_scalar_max(out=dur[:], in0=dur[:], scalar1=1)

            # --- features: fB[p, i*D:(i+1)*D] = feat[4p+i, :]
            fB_h = nc.alloc_sbuf_tensor("fB_raw", [P, IPB * D], f32)
            fB = fB_h
            nc.sync.dma_start(
                out=fB[:],
                in_=feats[:, :].rearrange("(p i) d -> p (i d)", p=P),
            )
            fBv = fB[:].rearrange("p (i d) -> p i d", i=IPB)

            # dummy ACT op: forces the activation table load into the preamble
            # instead of stalling the first raw-region scalar copy
            warm = sb.tile([P, 1], f32, tag="warm")
            nc.vector.memset(warm[:], 0.0)
            nc.scalar.copy(out=warm[:], in_=warm[:])

            # --- strict-lower-triangular ones [128, 128]: ltri[k, p] = (p > k)
            iota_f = sb.tile([P, P], i32, tag="iota_f")
            nc.gpsimd.iota(out=iota_f[:], pattern=[[1, P]], base=0, channel_multiplier=0)
            iota_p = sb.tile([P, 1], i32, tag="iota_p")
            nc.gpsimd.iota(out=iota_p[:], pattern=[[1, 1]], base=0, channel_multiplier=1)
            ltri = sb.tile([P, P], f32, tag="ltri")
            nc.vector.tensor_tensor(
                out=ltri[:],
                in0=iota_f[:],
                in1=iota_p[:, 0:1].to_broadcast([P, P]),
                op=Alu.is_gt,
            )

            # --- within-partition inclusive scan of the 4 durations
            scan = sb.tile([P, IPB], i32, tag="scan")
            nc.vector.tensor_tensor_scan(
                out=scan[:],
                data0=dur[:],
                data1=dur[:],
                initial=0.0,
                op0=Alu.add,
                op1=Alu.bypass,
            )

            # --- cross-partition exclusive prefix of per-partition totals
            sf = sb.tile([P, 1], f32, tag="sf")
            nc.vector.tensor_copy(out=sf[:], in_=scan[:, IPB - 1 : IPB])
            pre_ps = pp.tile([P, 1], f32, tag="pre_ps")
            nc.tensor.matmul(pre_ps[:], ltri[:], sf[:], start=True, stop=True)
            pre = sb.tile([P, 1], i32, tag="pre")
            nc.vector.tensor_copy(out=pre[:], in_=pre_ps[:])

            # exc[p, i] = global exclusive cumsum at phoneme 4p+i
            exc = sb.tile([P, IPB], i32, tag="exc")
            nc.vector.tensor_tensor(out=exc[:], in0=scan[:], in1=dur[:], op=Alu.subtract)
            nc.vector.tensor_tensor(
                out=exc[:], in0=exc[:], in1=pre[:, 0:1].to_broadcast([P, IPB]), op=Alu.add
            )

            # --- scatter offsets [128, 16], col ci*4+i: binary block of size
            # s at exc + (dur & ~(2s-1)), masked to OOB unless (dur & s).
            # Computed as wide [128, 16] ops against per-column constant tiles
            # (built early on Pool). Column class order: OFFS_ORDER.
            offs_h = nc.alloc_sbuf_tensor("offs_raw", [P, 16], i32)
            offs = offs_h[:]
            negm = sb.tile([P, 16], i32, tag="negm")
            smask = sb.tile([P, 16], i32, tag="smask")
            for ci, s_ in enumerate(OFFS_ORDER):
                cols = slice(ci * IPB, (ci + 1) * IPB)
                nc.gpsimd.iota(out=negm[:, cols], pattern=[[0, IPB]], base=-(2 * s_), channel_multiplier=0)
                nc.gpsimd.iota(out=smask[:, cols], pattern=[[0, IPB]], base=s_, channel_multiplier=0)
            dur16 = sb.tile([P, 16], i32, tag="dur16")
            nc.vector.tensor_copy(
                out=dur16[:].rearrange("p (a i) -> p a i", a=IPB),
                in_=dur[:, None, :].to_broadcast([P, IPB, IPB]),
            )
            exc16 = sb.tile([P, 16], i32, tag="exc16")
            nc.vector.tensor_copy(
                out=exc16[:].rearrange("p (a i) -> p a i", a=IPB),
                in_=exc[:, None, :].to_broadcast([P, IPB, IPB]),
            )
            m16 = sb.tile([P, 16], i32, tag="m16")
            nc.vector.tensor_tensor(out=offs[:, :], in0=dur16[:], in1=negm[:], op=Alu.bitwise_and)
            nc.vector.tensor_tensor(out=offs[:, :], in0=offs[:, :], in1=exc16[:], op=Alu.add)
            nc.vector.tensor_tensor(out=m16[:], in0=dur16[:], in1=smask[:], op=Alu.bitwise_and)
            nc.vector.tensor_scalar(
                out=m16[:], in0=m16[:], scalar1=0, scalar2=OOB, op0=Alu.is_equal, op1=Alu.mult
            )
            nc.vector.tensor_tensor(out=offs[:, :], in0=offs[:, :], in1=m16[:], op=Alu.add)

            # rep[p, i, c, :] = feat[4p+i, :] for c = 0..7; built in the RAW
            # region (below) so replication overlaps scatter descriptor-gen.
            rep_h = nc.alloc_sbuf_tensor("rep_raw", [P, IPB, 8, D], f32)
            rep = rep_h[:]
            # bounds registers for the raw-region scatters (Pool is idle here)
            bregs = {s_: nc.gpsimd.to_reg(MAX_LEN - s_) for s_ in BLKS}

    # --- RAW region. The TileContext epilogue barrier guarantees fB, dur and
    # offs are ready. Copies (DVE: phonemes 0-1, ACT: 2-3) run concurrently
    # with the Pool engine's descriptor generation for the copy-free 1-row
    # scatters; the remaining scatters gate on the copy semaphore. No WAW
    # waits between scatters: Q7 streams desc-gen, SDMA drains overlap.
    fBr = fB[:].rearrange("p (i d) -> p i d", i=IPB)
    # The TileContext epilogue ends with a Pool-engine semaphore-range CLEAR;
    # raw-region sem increments on other engines must not race it. Pool
    # publishes go_sem after its epilogue; DVE/ACT wait before their copies.
    nc.gpsimd.sem_inc(go_sem, 1)
    nc.vector.wait_ge(go_sem, 1)
    nc.scalar.wait_ge(go_sem, 1)
    for c in range(8):
        v = nc.vector.tensor_copy(out=rep[:, 0:2, c, :], in_=fBr[:, 0:2, :])
        if c < 6:
            a = nc.scalar.copy(out=rep[:, 2:4, c, :], in_=fBr[:, 2:4, :])
            if c == 5:
                a.then_inc(cpy_sem, 1)
    nc.vector.tensor_copy(out=rep[:, 2:4, 6, :], in_=fBr[:, 2:4, :])
    nc.vector.tensor_copy(out=rep[:, 2:4, 7, :], in_=fBr[:, 2:4, :]).then_inc(cpy_sem, 1)

    def scat(ci, i, src_ap, s_):
        c = ci * IPB + i
        nc.gpsimd.indirect_dma_start(
            out=out[:, :],
            out_offset=bass.IndirectOffsetOnAxis(ap=offs[:, c : c + 1], axis=0),
            in_=src_ap,
            in_offset=None,
            bounds_check=bregs[s_],
            oob_is_err=False,
        ).then_inc(scat_sem, 16)

    for i in range(IPB):  # 1-row blocks: source is fB, no copy dependency
        scat(0, i, fBr[:, i, :], 1)
    nc.gpsimd.wait_ge(cpy_sem, 2)
    for ci, s_ in ((1, 8), (2, 4), (3, 2)):
        for i in range(IPB):
            scat(ci, i, rep[:, i, 0:s_, :].rearrange("p c d -> p (c d)"), s_)
    nc.gpsimd.wait_ge(scat_sem, 16 * 16)

    nc.compile()
    return nc


def _get_nc():
    if "nc" not in _CACHE:
        _CACHE["nc"] = _build_nc()
    return _CACHE["nc"]


def _run(features, durations, trace=False):
    """features (B,T,D) f32, durations (B,T) i32 -> (out (B,MAX_LEN,D) f32, BassKernelResults)."""
    from concourse.bass_utils import run_bass_kernel_spmd

    nc = _get_nc()
    in_maps = []
    for b in range(B):
        in_maps.append(
            {
                "features": np.ascontiguousarray(features[b]),
                "durations": np.ascontiguousarray(durations[b].reshape(P, IPB)),
            }
        )
    kwargs = {}
    if trace:
        kwargs = dict(trace=True, trace_cores=list(range(B)), stitch_traces=False)
    res = run_bass_kernel_spmd(nc, in_maps, core_ids=list(range(B)), **kwargs)
    outs = np.stack([res.results[b]["out"] for b in range(B)])
    return outs.astype(np.float32, copy=False), res


def kernel(features, durations):
    features = np.asarray(features, dtype=np.float32)
    durations = np.asarray(durations, dtype=np.int32)
    outs, _ = _run(features, durations, trace=False)
    return outs


if __name__ == "__main__":
    feats = np.random.randn(B, T, D).astype(np.float32)
    durs = np.random.randint(0, 16, size=(B, T)).astype(np.int32)
    out = kernel(feats, durs)
    print("out", out.shape, out.dtype)
